# revision 6
# baseline (speedup 1.0000x reference)
"""Trainium2 Bass kernel for EpidemicDynamics: y = 0.1 * x * (A @ (1 - x)).

A is [16384, 16384] f32 (1 GiB) -> memory-bound matvec. Sharding: row-shard A
across 8 NeuronCores (contiguous [2048, 16384] slices), replicate x. Each core
computes its 2048 output rows locally; host concatenates. No collectives.

Per-core dataflow:
  - broadcast-DMA x into [128, 16384] (every partition holds the full vector),
    one fused tensor_scalar computes w = 1 - x in place.
  - stream the A slice as 32 tiles of [128 rows, 8192 cols] (4 MiB DMAs);
    for each tile one DVE tensor_tensor_reduce computes
    acc[p] = sum_f(A[p, f] * w[f]) in a single pass (product goes to a
    dummy free-step-0 AP, so no full-size scratch is needed).
  - final: y = (x_rows * 0.1) * acc via one scalar_tensor_tensor.
"""

import numpy as np

import concourse.bacc as bacc
import concourse.bass as bass
import concourse.mybir as mybir
import concourse.tile as tile
from concourse.bass_utils import run_bass_kernel_spmd

N = 16384          # problem size (hardcoded per harness contract)
NCORES = 8
ROWS = N // NCORES  # 2048 rows per core
P = 128             # SBUF partitions
NT = ROWS // P      # 16 row-tiles per core
CHUNK = 8192        # columns per tile
NCH = N // CHUNK    # 2 chunks per row-tile
R_COEF = 0.1

F32 = mybir.dt.float32


def build():
    nc = bacc.Bacc()
    A_s = nc.declare_dram_parameter("A_s", [ROWS, N], F32, isOutput=False)
    x_full = nc.declare_dram_parameter("x_full", [N, 1], F32, isOutput=False)
    x_s = nc.declare_dram_parameter("x_s", [ROWS, 1], F32, isOutput=False)
    y_s = nc.declare_dram_parameter("y_s", [ROWS, 1], F32, isOutput=True)

    with tile.TileContext(nc) as tc:
        with (
            tc.tile_pool(name="singles", bufs=1) as singles,
            tc.tile_pool(name="apool", bufs=3) as apool,
        ):
            # x replicated to all partitions, then w = 1 - x in place.
            w_rep = singles.tile([P, N], F32)
            x_row = x_full.rearrange("n o -> o n")  # [1, N]
            nc.sync.dma_start(out=w_rep[:], in_=x_row.to_broadcast([P, N]))
            nc.vector.tensor_scalar(
                w_rep[:], w_rep[:], -1.0, 1.0,
                mybir.AluOpType.mult, mybir.AluOpType.add,
            )

            # x rows for this core, laid out [p, t] to match acc.
            x_sb = singles.tile([P, NT], F32)
            nc.sync.dma_start(
                out=x_sb[:], in_=x_s.rearrange("(t p) o -> p t", p=P)
            )

            acc = singles.tile([P, NT * NCH], F32)
            dummy = singles.tile([P, 1], F32)

            for t in range(NT):
                for c in range(NCH):
                    at = apool.tile([P, CHUNK], F32, tag="A")
                    nc.sync.dma_start(
                        out=at[:],
                        in_=A_s[t * P:(t + 1) * P, c * CHUNK:(c + 1) * CHUNK],
                    )
                    k = t * NCH + c
                    # acc[:, k] = sum_f (A * R) * w  (scale by R rides along)
                    nc.vector.scalar_tensor_tensor(
                        out=dummy.broadcast_to([P, CHUNK]),
                        in0=at[:],
                        scalar=R_COEF,
                        in1=w_rep[:, c * CHUNK:(c + 1) * CHUNK],
                        op0=mybir.AluOpType.mult,
                        op1=mybir.AluOpType.mult,
                        accum_out=acc[:, k:k + 1],
                    )

            # reduce the NCH partial sums per row-tile: [P, NT, NCH] -> [P, NT]
            red = singles.tile([P, NT], F32)
            nc.vector.tensor_reduce(
                red[:],
                acc.rearrange("p (t c) -> p t c", c=NCH),
                axis=mybir.AxisListType.X,
                op=mybir.AluOpType.add,
            )

            # y = x * acc  (R already folded into the accumulation)
            y_sb = singles.tile([P, NT], F32)
            nc.vector.tensor_tensor(
                y_sb[:], x_sb[:], red[:], mybir.AluOpType.mult
            )
            nc.sync.dma_start(
                out=y_s.rearrange("(t p) o -> p t", p=P), in_=y_sb[:]
            )
    nc.compile()
    return nc


_NC = None


def _get_nc():
    global _NC
    if _NC is None:
        _NC = build()
    return _NC


def _in_maps(x, A):
    return [
        {
            "A_s": A[c * ROWS:(c + 1) * ROWS],
            "x_full": x,
            "x_s": x[c * ROWS:(c + 1) * ROWS],
        }
        for c in range(NCORES)
    ]


def run(t, x, A, **kw):
    """Run on the 8 NeuronCores; returns (y, BassKernelResults)."""
    x = np.ascontiguousarray(np.asarray(x, dtype=np.float32).reshape(N, 1))
    A = np.asarray(A, dtype=np.float32)
    res = run_bass_kernel_spmd(
        _get_nc(), _in_maps(x, A), list(range(NCORES)), **kw
    )
    y = np.concatenate(
        [np.asarray(res.results[c]["y_s"]) for c in range(NCORES)], axis=0
    )
    return y.astype(np.float32), res


def kernel(t, x, A):
    y, _ = run(t, x, A)
    return y


# revision 7
# speedup vs baseline: 1.0195x; 1.0195x over previous
"""Trainium2 Bass kernel for EpidemicDynamics: y = 0.1 * x * (A @ (1 - x)).

A is [16384, 16384] f32 (1 GiB) -> memory-bound matvec. Sharding: row-shard A
across 8 NeuronCores (contiguous [2048, 16384] slices), replicate x. Each core
computes its 2048 output rows locally; host concatenates. No collectives.

Per-core dataflow (v2):
  - x arrives once as a [1, 16384] row (64 KiB DMA). A PE outer-product
    (ones[1,128].T @ x_chunk[1,512]) broadcasts it to all 128 partitions in
    PSUM, and ACT copies PSUM->SBUF fused with w = 1 - x. This avoids an
    8.4 MB broadcast read from HBM.
  - partition p owns rows p*16 + t (t=0..15), so the per-row x/y vectors are
    contiguous 64 B runs per partition (cheap DMA descriptors).
  - the A slice streams as 64 tiles of [128 rows, 4096 cols] (2 MiB DMAs);
    each tile takes one DVE scalar_tensor_tensor: product (A * R) * w written
    to a free-step-0 dummy, accum_out = per-partition row sum.
  - finale: y = x * acc (R already folded in), via small DVE ops.
"""

import numpy as np

import concourse.bacc as bacc
import concourse.bass as bass
import concourse.mybir as mybir
import concourse.tile as tile
from concourse.bass_utils import run_bass_kernel_spmd

N = 16384          # problem size (hardcoded per harness contract)
NCORES = 8
ROWS = N // NCORES  # 2048 rows per core
P = 128             # SBUF partitions
NT = ROWS // P      # 16 rows per partition
CHUNK = 4096        # columns per A tile
NCH = N // CHUNK    # 4 chunks per row group
BC = 512            # broadcast piece (one PSUM bank)
XP = 4096           # x row piece held in SBUF
R_COEF = 0.1

F32 = mybir.dt.float32


def build():
    nc = bacc.Bacc()
    A_s = nc.declare_dram_parameter("A_s", [ROWS, N], F32, isOutput=False)
    x_full = nc.declare_dram_parameter("x_full", [N, 1], F32, isOutput=False)
    x_s = nc.declare_dram_parameter("x_s", [ROWS, 1], F32, isOutput=False)
    y_s = nc.declare_dram_parameter("y_s", [ROWS, 1], F32, isOutput=True)

    # partition p <-> rows p*NT + t: [128, CHUNK] tiles with row stride NT*N
    A_r = A_s.rearrange("(p t) n -> t p n", t=NT)
    x_row = x_full.rearrange("n o -> o n")  # [1, N]

    with tile.TileContext(nc) as tc:
        with (
            tc.tile_pool(name="singles", bufs=1) as singles,
            tc.tile_pool(name="xrow", bufs=2) as xrow_pool,
            tc.tile_pool(name="apool", bufs=5) as apool,
            tc.tile_pool(name="psum", bufs=4, space="PSUM") as psum_pool,
        ):
            ones = singles.tile([1, P], F32)
            nc.vector.memset(ones[:], 1.0)

            # w = 1 - x, replicated on all partitions, built on PE + ACT.
            w_rep = singles.tile([P, N], F32)
            for piece in range(N // XP):
                xp = xrow_pool.tile([1, XP], F32, tag="xr")
                nc.sync.dma_start(
                    out=xp[:], in_=x_row[:, piece * XP:(piece + 1) * XP]
                )
                for j in range(XP // BC):
                    col = piece * XP + j * BC
                    ps = psum_pool.tile([P, BC], F32, tag="bc")
                    nc.tensor.matmul(
                        ps[:],
                        ones[:],
                        xp[:, j * BC:(j + 1) * BC],
                        start=True,
                        stop=True,
                    )
                    nc.scalar.activation(
                        w_rep[:, col:col + BC],
                        ps[:],
                        mybir.ActivationFunctionType.Identity,
                        bias=1.0,
                        scale=-1.0,
                    )

            # x rows for this core: partition p gets x[p*NT:(p+1)*NT] (64 B).
            x_sb = singles.tile([P, NT], F32)
            nc.sync.dma_start(
                out=x_sb[:], in_=x_s.rearrange("(p t) o -> p (t o)", t=NT)
            )

            acc = singles.tile([P, NT * NCH], F32)
            dummy = singles.tile([P, 1], F32)

            for t in range(NT):
                for c in range(NCH):
                    at = apool.tile([P, CHUNK], F32, tag="A")
                    nc.sync.dma_start(
                        out=at[:],
                        in_=A_r[t, :, c * CHUNK:(c + 1) * CHUNK],
                    )
                    k = t * NCH + c
                    # acc[:, k] = sum_f (A * R) * w  (scale by R rides along)
                    nc.vector.scalar_tensor_tensor(
                        out=dummy.broadcast_to([P, CHUNK]),
                        in0=at[:],
                        scalar=R_COEF,
                        in1=w_rep[:, c * CHUNK:(c + 1) * CHUNK],
                        op0=mybir.AluOpType.mult,
                        op1=mybir.AluOpType.mult,
                        accum_out=acc[:, k:k + 1],
                    )

            # reduce the NCH partial sums per row: [P, NT, NCH] -> [P, NT]
            red = singles.tile([P, NT], F32)
            nc.vector.tensor_reduce(
                red[:],
                acc.rearrange("p (t c) -> p t c", c=NCH),
                axis=mybir.AxisListType.X,
                op=mybir.AluOpType.add,
            )

            # y = x * acc  (R already folded into the accumulation)
            y_sb = singles.tile([P, NT], F32)
            nc.vector.tensor_tensor(
                y_sb[:], x_sb[:], red[:], mybir.AluOpType.mult
            )
            nc.sync.dma_start(
                out=y_s.rearrange("(p t) o -> p (t o)", t=NT), in_=y_sb[:]
            )
    nc.compile()
    return nc


_NC = None


def _get_nc():
    global _NC
    if _NC is None:
        _NC = build()
    return _NC


def _in_maps(x, A):
    return [
        {
            "A_s": A[c * ROWS:(c + 1) * ROWS],
            "x_full": x,
            "x_s": x[c * ROWS:(c + 1) * ROWS],
        }
        for c in range(NCORES)
    ]


def run(t, x, A, **kw):
    """Run on the 8 NeuronCores; returns (y, BassKernelResults)."""
    x = np.ascontiguousarray(np.asarray(x, dtype=np.float32).reshape(N, 1))
    A = np.asarray(A, dtype=np.float32)
    res = run_bass_kernel_spmd(
        _get_nc(), _in_maps(x, A), list(range(NCORES)), **kw
    )
    y = np.concatenate(
        [np.asarray(res.results[c]["y_s"]) for c in range(NCORES)], axis=0
    )
    return y.astype(np.float32), res


def kernel(t, x, A):
    y, _ = run(t, x, A)
    return y


# revision 9
# speedup vs baseline: 1.2609x; 1.2368x over previous
"""Trainium2 Bass kernel for EpidemicDynamics: y = 0.1 * x * (A @ (1 - x)).

A is [16384, 16384] f32 (1 GiB) -> memory-bound matvec. Sharding: row-shard A
across 8 NeuronCores (contiguous [2048, 16384] slices), replicate x. Each core
computes its 2048 output rows locally; host concatenates. No collectives.

Per-core dataflow (v2):
  - x arrives once as a [1, 16384] row (64 KiB DMA). A PE outer-product
    (ones[1,128].T @ x_chunk[1,512]) broadcasts it to all 128 partitions in
    PSUM, and ACT copies PSUM->SBUF fused with w = 1 - x. This avoids an
    8.4 MB broadcast read from HBM.
  - partition p owns rows p*16 + t (t=0..15), so the per-row x/y vectors are
    contiguous 64 B runs per partition (cheap DMA descriptors).
  - the A slice streams as 64 tiles of [128 rows, 4096 cols] (2 MiB DMAs);
    each tile takes one DVE scalar_tensor_tensor: product (A * R) * w written
    to a free-step-0 dummy, accum_out = per-partition row sum.
  - finale: y = x * acc (R already folded in), via small DVE ops.
"""

import numpy as np

import concourse.bacc as bacc
import concourse.bass as bass
import concourse.mybir as mybir
import concourse.tile as tile
from concourse.bass_utils import run_bass_kernel_spmd

N = 16384          # problem size (hardcoded per harness contract)
NCORES = 8
ROWS = N // NCORES  # 2048 rows per core
P = 128             # SBUF partitions
NT = ROWS // P      # 16 rows per partition
CHUNK = 4096        # columns per A tile
NCH = N // CHUNK    # 4 chunks per row group
BC = 512            # broadcast piece (one PSUM bank)
XP = 4096           # x row piece held in SBUF
R_COEF = 0.1

F32 = mybir.dt.float32


def build():
    nc = bacc.Bacc()
    A_s = nc.declare_dram_parameter("A_s", [ROWS, N], F32, isOutput=False)
    x_full = nc.declare_dram_parameter("x_full", [N, 1], F32, isOutput=False)
    x_s = nc.declare_dram_parameter("x_s", [ROWS, 1], F32, isOutput=False)
    y_s = nc.declare_dram_parameter("y_s", [ROWS, 1], F32, isOutput=True)

    # partition p <-> rows p*NT + t: [128, CHUNK] tiles with row stride NT*N
    A_r = A_s.rearrange("(p t) n -> t p n", t=NT)
    x_row = x_full.rearrange("n o -> o n")  # [1, N]

    with tile.TileContext(nc) as tc:
        with (
            tc.tile_pool(name="singles", bufs=1) as singles,
            tc.tile_pool(name="xrow", bufs=2) as xrow_pool,
            tc.tile_pool(name="apool", bufs=5) as apool,
            tc.tile_pool(name="psum", bufs=4, space="PSUM") as psum_pool,
        ):
            ones = singles.tile([1, P], F32)
            nc.vector.memset(ones[:], 1.0)

            # w = 1 - x, replicated on all partitions, built on PE + ACT.
            # One tile per CHUNK of columns so consumers wait only on their
            # own piece, not the whole 16K vector.
            w_tiles = []
            for piece in range(N // XP):
                xp = xrow_pool.tile([1, XP], F32, tag="xr")
                nc.sync.dma_start(
                    out=xp[:], in_=x_row[:, piece * XP:(piece + 1) * XP]
                )
                wt = singles.tile([P, XP], F32, tag=f"w{piece}")
                w_tiles.append(wt)
                for j in range(XP // BC):
                    ps = psum_pool.tile([P, BC], F32, tag="bc")
                    nc.tensor.matmul(
                        ps[:],
                        ones[:],
                        xp[:, j * BC:(j + 1) * BC],
                        start=True,
                        stop=True,
                    )
                    nc.scalar.activation(
                        wt[:, j * BC:(j + 1) * BC],
                        ps[:],
                        mybir.ActivationFunctionType.Identity,
                        bias=1.0,
                        scale=-1.0,
                    )

            # x rows for this core: partition p gets x[p*NT:(p+1)*NT] (64 B).
            x_sb = singles.tile([P, NT], F32)
            nc.sync.dma_start(
                out=x_sb[:], in_=x_s.rearrange("(p t) o -> p (t o)", t=NT)
            )

            acc = singles.tile([P, NT * NCH], F32)
            dummy = singles.tile([P, 1], F32)

            # column-major: all row groups' chunk c before chunk c+1, so the
            # first 16 DVE ops need only w_tiles[0] (ready earliest).
            for c in range(NCH):
                for t in range(NT):
                    at = apool.tile([P, CHUNK], F32, tag="A")
                    nc.sync.dma_start(
                        out=at[:],
                        in_=A_r[t, :, c * CHUNK:(c + 1) * CHUNK],
                    )
                    k = t * NCH + c
                    # acc[:, k] = sum_f (A * R) * w  (scale by R rides along)
                    nc.vector.scalar_tensor_tensor(
                        out=dummy.broadcast_to([P, CHUNK]),
                        in0=at[:],
                        scalar=R_COEF,
                        in1=w_tiles[c][:],
                        op0=mybir.AluOpType.mult,
                        op1=mybir.AluOpType.mult,
                        accum_out=acc[:, k:k + 1],
                    )

            # reduce the NCH partial sums per row: [P, NT, NCH] -> [P, NT]
            red = singles.tile([P, NT], F32)
            nc.vector.tensor_reduce(
                red[:],
                acc.rearrange("p (t c) -> p t c", c=NCH),
                axis=mybir.AxisListType.X,
                op=mybir.AluOpType.add,
            )

            # y = x * acc  (R already folded into the accumulation)
            y_sb = singles.tile([P, NT], F32)
            nc.vector.tensor_tensor(
                y_sb[:], x_sb[:], red[:], mybir.AluOpType.mult
            )
            nc.sync.dma_start(
                out=y_s.rearrange("(p t) o -> p (t o)", t=NT), in_=y_sb[:]
            )
    nc.compile()
    return nc


_NC = None


def _get_nc():
    global _NC
    if _NC is None:
        _NC = build()
    return _NC


def _in_maps(x, A):
    return [
        {
            "A_s": A[c * ROWS:(c + 1) * ROWS],
            "x_full": x,
            "x_s": x[c * ROWS:(c + 1) * ROWS],
        }
        for c in range(NCORES)
    ]


def run(t, x, A, **kw):
    """Run on the 8 NeuronCores; returns (y, BassKernelResults)."""
    x = np.ascontiguousarray(np.asarray(x, dtype=np.float32).reshape(N, 1))
    A = np.asarray(A, dtype=np.float32)
    res = run_bass_kernel_spmd(
        _get_nc(), _in_maps(x, A), list(range(NCORES)), **kw
    )
    y = np.concatenate(
        [np.asarray(res.results[c]["y_s"]) for c in range(NCORES)], axis=0
    )
    return y.astype(np.float32), res


def kernel(t, x, A):
    y, _ = run(t, x, A)
    return y
